# revision 51
# baseline (speedup 1.0000x reference)
"""Trainium2 Bass kernel for nn_Correlation_Block (N=32, F=1024, D=512, H=1024).

Data-parallel over batch N across 8 NeuronCores (4 samples each).
BatchNorm batch statistics combined across cores with tiny AllReduces
(plus two dummy warmup AllReduces at kernel start to ramp the CC cores).

Host-side (free, not in HW exec time):
  - x passed twice as fp16: xh [fi,fo,d] and xT [di,do,f] (no device transposes)
  - weights passed fp16 pre-transposed: W0T [di,do,h],
    WcIT [ii,io,o] with (Wc + I) folded so t = conv(xr)+xr is one matmul chain
  - MT [di,do,d] = (W0^T W1^T) reshaped: z-tilde = x @ M collapses the
    K=1024 contraction of z = v1 @ W1^T to K=512 (half the PE work); the
    BN0 affine is restored with z = a0*zt + c0 x s1row (s1 = row sums of W1)
  - BN gamma/beta pre-tiled [P, FO]

Device-side per sample:
  v0 = x @ W0^T            (64 MMs, K=512) + bn stats
  zt = x @ M               (32 MMs, K=512)
  v0T                      (64 PE transposes, pre-affine, in Phase A ->
                            no v0 DRAM spill; they also fill the AR1 window)
  [AR1] -> a0/c0; aB/cB row-broadcast tiles built via PE transpose + DMA
  v1T = aB*v0T + cB        (DVE, broadcast affine along free dim)
  w  = softsign(v1 @ v1^T) (96 MMs using symmetry; 4 mirrored tiles)
  z  = a0*zt + c0*s1row    (DVE fixup in place)
  u  = w @ z               (64 MMs, sw used as lhsT via symmetry); u spills
                           to DRAM (SBUF is too tight to hold u + v1T + zt)
  [AR2] -> merged BN1+feed_norm affines
  xr = A*u + af*x + Cc     (ACT + fused DVE stt)
  t  = (Wc+I) @ xr (+bc)   (64 MMs in two 4-channel waves across samples;
                            each wave's stats AllReduce + BN apply + output
                            DMA overlap the other wave's matmuls)
"""

import numpy as np

N, F, D = 32, 1024, 512
H = 1024
NCORES = 8
NS = N // NCORES          # samples per core
EPS = 1e-5
P = 128
FO = F // P               # 8 f-chunks
DO = D // P               # 4 d-chunks
HO = H // P               # 8 h-chunks
HH = H // 512             # 2 (512-wide halves of H)

_CACHE = {}

import os
STAGE = int(os.environ.get("BASS_STAGE", "99"))
DEBUG = int(os.environ.get("BASS_DEBUG", "0"))


def _build(has_bias):
    import concourse.bass as bass
    import concourse.tile as tile
    from concourse import bacc, mybir
    from concourse.masks import make_identity

    f32 = mybir.dt.float32
    f16 = mybir.dt.float16

    nc = bacc.Bacc("TRN2", target_bir_lowering=False, debug=False,
                   num_devices=NCORES)

    # ---- I/O ----
    xh_io = nc.dram_tensor("xh", [NS, P, FO, D], f16, kind="ExternalInput").ap()
    xT_io = nc.dram_tensor("xT", [NS, P, DO, F], f16, kind="ExternalInput").ap()
    W0T_io = nc.dram_tensor("W0T", [P, DO, H], f16, kind="ExternalInput").ap()
    MT_io = nc.dram_tensor("MT", [P, DO, D], f16, kind="ExternalInput").ap()
    s1r_io = nc.dram_tensor("s1r", [P, D], f16, kind="ExternalInput").ap()
    WcIT_io = nc.dram_tensor("WcIT", [P, FO, F], f16, kind="ExternalInput").ap()
    prm_io = nc.dram_tensor("prm", [P, 9, FO], f32, kind="ExternalInput").ap()
    if has_bias:
        b0r_io = nc.dram_tensor("b0r", [1, H], f32, kind="ExternalInput").ap()
        b1r_io = nc.dram_tensor("b1r", [1, D], f32, kind="ExternalInput").ap()
    out_io = nc.dram_tensor("out", [NS, F, D], f32, kind="ExternalOutput").ap()
    if DEBUG:
        dbg_ac = nc.dram_tensor("dbg_ac", [P, 4, F], f32,
                                kind="ExternalOutput").ap()
        dbg_vT = nc.dram_tensor("dbg_vT", [P, HO, F], f32,
                                kind="ExternalOutput").ap()
        dbg_zt = nc.dram_tensor("dbg_zt", [P, FO, D], f32,
                                kind="ExternalOutput").ap()
        dbg_sw = nc.dram_tensor("dbg_sw", [P, FO, F], f32,
                                kind="ExternalOutput").ap()
        dbg_u = nc.dram_tensor("dbg_u", [P, FO, D], f32,
                               kind="ExternalOutput").ap()
        dbg_xr = nc.dram_tensor("dbg_xr", [P, FO, D], f32,
                                kind="ExternalOutput").ap()

    add = mybir.AluOpType.add
    sub = mybir.AluOpType.subtract
    mult = mybir.AluOpType.mult
    Ident = mybir.ActivationFunctionType.Identity
    Copy = mybir.ActivationFunctionType.Copy
    Sqrt = mybir.ActivationFunctionType.Sqrt
    Abs = mybir.ActivationFunctionType.Abs
    AXX = mybir.AxisListType.X

    with tile.TileContext(nc) as tc:
        with tc.tile_pool(name="persist", bufs=1) as persist, \
             tc.tile_pool(name="xh", bufs=2) as xh_pool, \
             tc.tile_pool(name="ut", bufs=2) as ut_pool, \
             tc.tile_pool(name="small", bufs=1) as small, \
             tc.tile_pool(name="pmm", bufs=6, space="PSUM") as pmm, \
             tc.tile_pool(name="ptr", bufs=2, space="PSUM") as ptr, \
             tc.tile_pool(name="dram", bufs=1, space="DRAM") as dram:

            # ---- three chained dummy AllReduces (64KB): absorb CC-core
            # boot (~50us) and keep the CC busy/warm until AR1 ----
            ar0 = small.tile([P, 128], f32)
            nc.vector.memset(ar0[:], 1.0)
            ar0_in = dram.tile([P, 128], f32, tag="ar0_in")
            ar0_out = dram.tile([P, 128], f32, tag="ar0_out")
            ar0b_out = dram.tile([P, 128], f32, tag="ar0b_out")
            ar0c_out = dram.tile([P, 128], f32, tag="ar0c_out")
            nc.scalar.dma_start(ar0_in[:], ar0[:])
            nc.gpsimd.collective_compute(
                "AllReduce", add, replica_groups=[list(range(NCORES))],
                ins=[ar0_in.opt()], outs=[ar0_out.opt()])
            nc.gpsimd.collective_compute(
                "AllReduce", add, replica_groups=[list(range(NCORES))],
                ins=[ar0_out.opt()], outs=[ar0b_out.opt()])
            nc.gpsimd.collective_compute(
                "AllReduce", add, replica_groups=[list(range(NCORES))],
                ins=[ar0b_out.opt()], outs=[ar0c_out.opt()])
            # read-back deferred (a waiting DMA would block its queue)

            ident = persist.tile([P, P], f16)
            make_identity(nc, ident[:])

            eps_col = persist.tile([P, 1], f32)
            nc.vector.memset(eps_col[:], EPS)

            # ---- weights (host-prepared, straight DMA) ----
            # Weights ride the Activation HWDGE queue, x loads ride Sync.
            W0T = persist.tile([P, DO, H], f16)
            MT = persist.tile([P, DO, D], f16)
            s1r = persist.tile([P, D], f16)
            WcIT = persist.tile([P, FO, F], f16)
            prm = persist.tile([P, 9, FO], f32)
            gamma0 = prm[:, 0, :]
            beta0 = prm[:, 1, :]
            gamma1 = prm[:, 2, :]
            beta1 = prm[:, 3, :]
            gammaf = prm[:, 4, :]
            betaf = prm[:, 5, :]
            gammao = prm[:, 6, :]
            betao = prm[:, 7, :]
            bc_col = prm[:, 8, :]
            # per-dc split so the first matmul only waits for its own slice
            for dc in range(DO):
                nc.scalar.dma_start(W0T[:, dc, :], W0T_io[:, dc, :])
            nc.scalar.dma_start(MT[:], MT_io)

            if has_bias:
                xone = persist.tile([P, P], f16)
                nc.vector.memset(xone[:], 0.0)
                nc.vector.memset(xone[0:1, :], 1.0)
                W0b = persist.tile([P, H], f16)
                W1b = persist.tile([P, D], f16)
                nc.vector.memset(W0b[:], 0.0)
                nc.vector.memset(W1b[:], 0.0)
                with tc.tile_pool(name="btmp", bufs=2) as btmp:
                    t = btmp.tile([1, H], f32, tag="b")
                    nc.sync.dma_start(t[:], b0r_io)
                    nc.vector.tensor_copy(out=W0b[0:1, :], in_=t[:])
                    t = btmp.tile([1, D], f32, tag="b")
                    nc.sync.dma_start(t[:], b1r_io)
                    nc.vector.tensor_copy(out=W1b[0:1, :], in_=t[:])

            # stats slot tiles
            vslots = small.tile([P, FO, HH * NS, 6], f32)
            xslots = small.tile([P, FO, NS, 6], f32)
            uslots = small.tile([P, FO, NS, 6], f32)
            xuslots = small.tile([P, FO * NS], f32)
            tslots = small.tile([P, FO, NS, 6], f32)

            vT = []       # per-sample v0T (pre-affine) [P, HO, F]
            ztl = []      # per-sample z-tilde [P, FO, D]
            u_dram = []

            vT_zt_ctx = tc.tile_pool(name="vT", bufs=1)
            vT_pool = vT_zt_ctx.__enter__()
            zt_ctx = tc.tile_pool(name="zt", bufs=1)
            zt_pool = zt_ctx.__enter__()

            if STAGE >= 1:
                # ============ PHASE A ============
                # per sample: v0 = x@W0T (+stats), transposes, zt = x@M
                with tc.tile_pool(name="xT", bufs=2) as xT_pool, \
                     tc.tile_pool(name="v0sb", bufs=1) as v0_pool:
                    for s in range(NS):
                        xTs = xT_pool.tile([P, DO, F], f16, tag="xT")
                        if s == 0:
                            for dc in range(DO):
                                nc.sync.dma_start(xTs[:, dc, :],
                                                  xT_io[s, :, dc, :])
                        else:
                            nc.sync.dma_start(xTs[:], xT_io[s])
                        xhs = xh_pool.tile([P, FO, D], f16, tag="xh")
                        nc.sync.dma_start(xhs[:], xh_io[s])
                        if s == 0:
                            nc.scalar.dma_start(s1r[:], s1r_io)
                        if s == 1:
                            nc.scalar.dma_start(WcIT[:], WcIT_io)
                            nc.scalar.dma_start(prm[:], prm_io)
                        # xh stats first (they only need the DMA; keep them
                        # off the post-matmul DVE critical path)
                        for fo in range(FO):
                            nc.vector.bn_stats(out=xslots[:, fo, s, :],
                                               in_=xhs[:, fo, :])
                        vTs = vT_pool.tile([P, HO, F], f16, name=f"vT{s}")
                        vT.append(vTs)
                        # hh-split: compute the 512-wide h-half for all fc,
                        # transpose it, then the other half (v0sb is only one
                        # half-H wide to save SBUF)
                        for hh in range(HH):
                            v0sb = v0_pool.tile([P, FO, 512], f16, tag="v0")
                            for fc in range(FO):
                                pv = pmm.tile([P, 512], f32, tag="mm")
                                for dc in range(DO):
                                    nc.tensor.matmul(
                                        pv[:],
                                        lhsT=xTs[:, dc, fc * P:(fc + 1) * P],
                                        rhs=W0T[:, dc, hh * 512:(hh + 1) * 512],
                                        start=(dc == 0),
                                        stop=(dc == DO - 1 and not has_bias))
                                if has_bias:
                                    nc.tensor.matmul(
                                        pv[:], lhsT=xone[:],
                                        rhs=W0b[:, hh * 512:(hh + 1) * 512],
                                        start=False, stop=True)
                                nc.vector.bn_stats(
                                    out=vslots[:, fc, hh * NS + s, :], in_=pv[:])
                                nc.scalar.activation(
                                    v0sb[:, fc, :], pv[:], Copy)
                            # v0T transposes (pre-affine); the last sample's
                            # run inside the AR1 window
                            for hl in range(4):
                                ho = hh * 4 + hl
                                pt = ptr.tile([P, F], f16, tag="tr")
                                for fc in range(FO):
                                    nc.tensor.transpose(
                                        pt[:, fc * P:(fc + 1) * P],
                                        v0sb[:, fc, hl * P:(hl + 1) * P],
                                        ident[:])
                                nc.scalar.activation(vTs[:, ho, :], pt[:], Copy)
                        # zt = x @ M  (K=512; half the work of v1@W1T)
                        zts = zt_pool.tile([P, FO, D], f16, name=f"zt{s}")
                        ztl.append(zts)
                        for fc in range(FO):
                            pz = pmm.tile([P, 512], f32, tag="mm")
                            for dc in range(DO):
                                nc.tensor.matmul(
                                    pz[:],
                                    lhsT=xTs[:, dc, fc * P:(fc + 1) * P],
                                    rhs=MT[:, dc, :],
                                    start=(dc == 0), stop=(dc == DO - 1))
                            nc.vector.tensor_copy(out=zts[:, fc, :], in_=pz[:])

            if STAGE >= 2:
                # ---- aggregate + AllReduce 1 ----
                # Wide-op aggregation from the bn_stats 6-tuples (lanes:
                # cnt_e, mean_e, M2_e, cnt_o, mean_o, M2_o; 256 elements per
                # parity).  AR lanes: 0 S_v, 1 Q_v, 2 S_x, 3 Q_x where
                # S = sum of means, Q = 256*sum(mean^2) + sum(M2); the 256x
                # count factors fold into the post-AR scaling.
                ar1 = small.tile([P, 4, FO], f32, tag="ar1")
                NSL = HH * NS
                tv = small.tile([P, FO, NSL], f32, tag="tv")
                tx = small.tile([P, FO, NS], f32, tag="tx")

                def slot_reduce(slots, t, lane_off, nsl):
                    me = slots[:, :, :, 1]
                    mo = slots[:, :, :, 4]
                    t2 = small.tile([P, FO, nsl], f32, name=f"t2s_{lane_off}")
                    nc.vector.tensor_tensor(t[:], me, mo, add)
                    nc.vector.tensor_reduce(
                        out=ar1[:, lane_off + 0, :], in_=t[:], axis=AXX, op=add)
                    nc.vector.tensor_tensor(t[:], me, me, mult)
                    nc.vector.tensor_tensor(t2[:], mo, mo, mult)
                    nc.vector.tensor_tensor(t[:], t[:], t2[:], add)
                    nc.vector.tensor_reduce(
                        out=ar1[:, lane_off + 1, :], in_=t[:], axis=AXX, op=add)
                    nc.vector.tensor_tensor(t[:], slots[:, :, :, 2],
                                            slots[:, :, :, 5], add)
                    nc.vector.tensor_reduce(
                        out=t2[:, :, 0:1], in_=t[:], axis=AXX, op=add)
                    nc.vector.scalar_tensor_tensor(
                        out=ar1[:, lane_off + 1, :],
                        in0=ar1[:, lane_off + 1, :], scalar=256.0,
                        in1=t2[:, :, 0], op0=mult, op1=add)

                slot_reduce(vslots, tv, 0, NSL)
                slot_reduce(xslots, tx, 2, NS)

                ar1_in = dram.tile([P, 4 * FO], f32, tag="ar1_in")
                ar1_out = dram.tile([P, 4 * FO], f32, tag="ar1_out")
                nc.scalar.dma_start(ar1_in[:], ar1[:].rearrange("p a b -> p (a b)"))
                nc.gpsimd.collective_compute(
                    "AllReduce", add, replica_groups=[list(range(NCORES))],
                    ins=[ar1_in.opt()], outs=[ar1_out.opt()])
                gsb1 = small.tile([P, 4, FO], f32, tag="gsb1")
                nc.scalar.dma_start(gsb1[:].rearrange("p a b -> p (a b)"),
                                    ar1_out[:])
                # deferred dummy-AR read-back (long done; software queue)
                ar0_back = small.tile([P, 128], f32, tag="ar0b")
                nc.gpsimd.dma_start(ar0_back[:], ar0c_out[:])

                def affine_from(mean_t, e2_t, gamma_t, beta_t, nm, w=FO):
                    """returns (a, c) tiles [P, w]"""
                    var_t = small.tile([P, w], f32, name=f"var_{nm}")
                    t2 = small.tile([P, w], f32, name=f"t2_{nm}")
                    nc.vector.tensor_tensor(t2[:], mean_t[:], mean_t[:], mult)
                    nc.vector.tensor_tensor(var_t[:], e2_t[:], t2[:], sub)
                    sd = small.tile([P, w], f32, name=f"sd_{nm}")
                    nc.scalar.activation(sd[:], var_t[:], Sqrt,
                                         bias=eps_col[:], scale=1.0)
                    nc.vector.reciprocal(sd[:], sd[:])
                    a_t = small.tile([P, w], f32, name=f"a_{nm}")
                    c_t = small.tile([P, w], f32, name=f"c_{nm}")
                    nc.vector.tensor_tensor(a_t[:], gamma_t[:], sd[:], mult)
                    nc.vector.tensor_tensor(t2[:], mean_t[:], a_t[:], mult)
                    nc.vector.tensor_tensor(c_t[:], beta_t[:], t2[:], sub)
                    return a_t, c_t

                # m = 256*S/(N*H); E2 = Q/(N*H)
                m0 = small.tile([P, FO], f32, tag="m0")
                e20 = small.tile([P, FO], f32, tag="e20")
                nc.vector.tensor_scalar_mul(m0[:], gsb1[:, 0, :],
                                            256.0 / (N * H))
                nc.vector.tensor_scalar_mul(e20[:], gsb1[:, 1, :],
                                            1.0 / (N * H))
                a0, c0 = affine_from(m0, e20, gamma0, beta0, "bn0")
                mx = small.tile([P, FO], f32, tag="mx")
                e2x = small.tile([P, FO], f32, tag="e2x")
                nc.vector.tensor_scalar_mul(mx[:], gsb1[:, 2, :],
                                            256.0 / (N * D))
                nc.vector.tensor_scalar_mul(e2x[:], gsb1[:, 3, :],
                                            1.0 / (N * D))

                # ---- aB/cB: broadcast a0/c0 along partitions as [P, F]
                # rows (f = fo*128+fi on the free axis).  Per fo-block:
                # replicate the per-partition coefficient column along free
                # (tensor_scalar by ones), then PE-transpose the block. ----
                ones128 = small.tile([P, P], f16, tag="ones128")
                nc.vector.memset(ones128[:], 1.0)
                acB = persist.tile([P, 2, F], f16)
                for lane, coef in enumerate([a0, c0]):
                    rep = small.tile([P, F], f16, name=f"rep{lane}")
                    for fo in range(FO):
                        nc.vector.tensor_scalar(
                            out=rep[:, fo * P:(fo + 1) * P], in0=ones128[:],
                            scalar1=coef[:, fo:fo + 1], scalar2=None, op0=mult)
                    pb = ptr.tile([P, F], f16, tag="tr")
                    for fo in range(FO):
                        nc.tensor.transpose(
                            pb[:, fo * P:(fo + 1) * P],
                            rep[:, fo * P:(fo + 1) * P], ident[:])
                    nc.scalar.activation(acB[:, lane, :], pb[:], Copy)
                aB = acB[:, 0, :]
                cB = acB[:, 1, :]
                if DEBUG:
                    with tc.tile_pool(name="dbgp", bufs=1) as dbgp:
                        st = dbgp.tile([P, F], f32, tag="st")
                        nc.vector.tensor_copy(out=st[:], in_=aB)
                        nc.sync.dma_start(dbg_ac[:, 0, :], st[:])
                        st = dbgp.tile([P, F], f32, tag="st")
                        nc.vector.tensor_copy(out=st[:], in_=cB)
                        nc.sync.dma_start(dbg_ac[:, 1, :], st[:])
                        st = dbgp.tile([P, F], f32, tag="st")
                        nc.vector.memset(st[:], 0.0)
                        nc.vector.tensor_copy(out=st[:, 0:FO], in_=a0[:])
                        nc.vector.tensor_copy(out=st[:, FO:2 * FO], in_=c0[:])
                        nc.vector.tensor_copy(out=st[:, 2 * FO:3 * FO],
                                              in_=m0[:])
                        nc.vector.tensor_copy(out=st[:, 3 * FO:4 * FO],
                                              in_=e20[:])
                        nc.sync.dma_start(dbg_ac[:, 2, :], st[:])

            if STAGE >= 3:
                # ============ PHASE B: w, z-fixup, u ============
                with tc.tile_pool(name="phb", bufs=1) as phb, \
                     tc.tile_pool(name="phbs", bufs=2) as phbs:
                    xh_b = {}

                    def prep(s):
                        # v1T = aB*v0T + cB (broadcast affine, in place);
                        # z = a0*zt + c0*s1row (in place); xh reload.
                        # Called one sample ahead (during sample s-1's
                        # u-matmuls) so the next w-matmuls never wait on DVE.
                        for ho in range(HO):
                            nc.vector.tensor_tensor(
                                vT[s][:, ho, :], vT[s][:, ho, :], aB, mult)
                            nc.vector.tensor_tensor(
                                vT[s][:, ho, :], vT[s][:, ho, :], cB, add)
                        for fc in range(FO):
                            nc.vector.tensor_scalar(
                                out=ztl[s][:, fc, :], in0=ztl[s][:, fc, :],
                                scalar1=a0[:, fc:fc + 1], scalar2=None,
                                op0=mult)
                            nc.vector.scalar_tensor_tensor(
                                out=ztl[s][:, fc, :], in0=s1r[:],
                                scalar=c0[:, fc:fc + 1], in1=ztl[s][:, fc, :],
                                op0=mult, op1=add)
                        xhs = xh_pool.tile([P, FO, D], f16, tag="xh")
                        nc.sync.dma_start(xhs[:], xh_io[s])
                        xh_b[s] = xhs

                    prep(0)
                    for s in range(NS):
                        xhs = xh_b[s]
                        if DEBUG and s == 0:
                            with tc.tile_pool(name="dbgb", bufs=1) as dbgb:
                                for ho in range(HO):
                                    st = dbgb.tile([P, F], f32, tag="st")
                                    nc.vector.tensor_copy(
                                        out=st[:], in_=vT[s][:, ho, :])
                                    nc.sync.dma_start(dbg_vT[:, ho, :], st[:])
                                for fc in range(FO):
                                    st = dbgb.tile([P, F], f32, tag="st")
                                    nc.vector.tensor_copy(
                                        out=st[:, 0:D], in_=ztl[s][:, fc, :])
                                    nc.sync.dma_start(dbg_zt[:, fc, :],
                                                      st[:, 0:D])

                        # w = softsign(v1 @ v1^T), exploiting symmetry
                        swsb = phb.tile([P, FO, F], f16, tag="sw")

                        def w_tile(fc, gg):
                            pw = pmm.tile([P, 512], f32, tag="mm")
                            for ho in range(HO):
                                nc.tensor.matmul(
                                    pw[:],
                                    lhsT=vT[s][:, ho, fc * P:(fc + 1) * P],
                                    rhs=vT[s][:, ho, gg * 512:(gg + 1) * 512],
                                    start=(ho == 0), stop=(ho == HO - 1))
                            absw = phbs.tile([P, 512], f32, tag="absw")
                            nc.scalar.activation(absw[:], pw[:], Abs)
                            nc.scalar.add(absw[:], absw[:], 1.0)
                            rcp = phbs.tile([P, 512], f32, tag="rcp")
                            nc.vector.reciprocal_approx_fast(rcp[:], absw[:])
                            nc.vector.tensor_tensor(
                                swsb[:, fc, gg * 512:(gg + 1) * 512],
                                pw[:], rcp[:], mult)

                        for fc in range(4):
                            for gg in range(HH):
                                w_tile(fc, gg)
                        # mirrors: swsb[:, fc4, rc*P:+P] =
                        #   transpose(swsb[:, rc, fc4*P:+P]) for rc 0..3
                        for fc4 in range(4, 8):
                            pt2 = ptr.tile([P, 512], f16, tag="tr")
                            for rc in range(4):
                                nc.tensor.transpose(
                                    pt2[:, rc * P:(rc + 1) * P],
                                    swsb[:, rc, fc4 * P:(fc4 + 1) * P],
                                    ident[:])
                            nc.scalar.activation(
                                swsb[:, fc4, 0:512], pt2[:], Copy)
                        for fc in range(4, 8):
                            w_tile(fc, 1)

                        # prep the NEXT sample now: its DVE work runs during
                        # this sample's u-matmuls
                        if s + 1 < NS:
                            prep(s + 1)

                        # u = w @ z  (sw as lhsT via symmetry)
                        us = ut_pool.tile([P, FO, D], f16, tag="ut")
                        for fc in range(FO):
                            pu = pmm.tile([P, 512], f32, tag="mm")
                            for gc in range(FO):
                                nc.tensor.matmul(
                                    pu[:],
                                    lhsT=swsb[:, gc, fc * P:(fc + 1) * P],
                                    rhs=ztl[s][:, gc, :],
                                    start=(gc == 0),
                                    stop=(gc == FO - 1 and not has_bias))
                            if has_bias:
                                nc.tensor.matmul(
                                    pu[:], lhsT=xone[:], rhs=W1b[:],
                                    start=False, stop=True)
                            nc.vector.bn_stats(out=uslots[:, fc, s, :], in_=pu[:])
                            junk = phbs.tile([P, 512], f32, tag="junk")
                            nc.vector.tensor_tensor(
                                junk[:], pu[:], xhs[:, fc, :], mult)
                            nc.vector.tensor_reduce(
                                out=xuslots[:, fc * NS + s:fc * NS + s + 1],
                                in_=junk[:], axis=AXX, op=add)
                            nc.scalar.activation(us[:, fc, :], pu[:], Copy)
                        ud = dram.tile([P, FO, D], f16, tag=f"ud{s}")
                        u_dram.append(ud)
                        nc.sync.dma_start(ud[:], us[:])
                        if DEBUG and s == 0:
                            with tc.tile_pool(name="dbgu", bufs=1) as dbgu:
                                for fc in range(FO):
                                    st = dbgu.tile([P, F], f32, tag="st")
                                    nc.vector.tensor_copy(
                                        out=st[:], in_=swsb[:, fc, :])
                                    nc.sync.dma_start(dbg_sw[:, fc, :], st[:])
                                    st = dbgu.tile([P, F], f32, tag="st")
                                    nc.vector.tensor_copy(
                                        out=st[:, 0:D], in_=us[:, fc, :])
                                    nc.sync.dma_start(dbg_u[:, fc, :],
                                                      st[:, 0:D])

            # pre-issue u reloads for the first two Phase-C samples so they
            # run during AR2
            u_pre = []
            if STAGE >= 5:
                for s in range(min(2, NS)):
                    upb = ut_pool.tile([P, FO, D], f16, tag="ut",
                                       name=f"upre{s}")
                    nc.sync.dma_start(upb[:], u_dram[s][:])
                    u_pre.append(upb)

            if STAGE >= 4:
                # ---- aggregate + AllReduce 2 ----
                # lanes: 0 S_u, 1 Q_u, 2 sum(x*u)
                ar2 = small.tile([P, 3, FO], f32, tag="ar2")
                tu = small.tile([P, FO, NS], f32, tag="tu")
                tu2 = small.tile([P, FO, NS], f32, tag="tu2")
                ume = uslots[:, :, :, 1]
                umo = uslots[:, :, :, 4]
                nc.vector.tensor_tensor(tu[:], ume, umo, add)
                nc.vector.tensor_reduce(out=ar2[:, 0, :], in_=tu[:],
                                        axis=AXX, op=add)
                nc.vector.tensor_tensor(tu[:], ume, ume, mult)
                nc.vector.tensor_tensor(tu2[:], umo, umo, mult)
                nc.vector.tensor_tensor(tu[:], tu[:], tu2[:], add)
                nc.vector.tensor_reduce(out=ar2[:, 1, :], in_=tu[:],
                                        axis=AXX, op=add)
                nc.vector.tensor_tensor(tu[:], uslots[:, :, :, 2],
                                        uslots[:, :, :, 5], add)
                nc.vector.tensor_reduce(out=tu2[:, :, 0:1], in_=tu[:],
                                        axis=AXX, op=add)
                nc.vector.scalar_tensor_tensor(
                    out=ar2[:, 1, :], in0=ar2[:, 1, :], scalar=256.0,
                    in1=tu2[:, :, 0], op0=mult, op1=add)
                nc.vector.tensor_reduce(
                    out=ar2[:, 2, :],
                    in_=xuslots[:].rearrange("p (fo s) -> p fo s", s=NS),
                    axis=AXX, op=add)

                ar2_in = dram.tile([P, 3 * FO], f32, tag="ar2_in")
                ar2_out = dram.tile([P, 3 * FO], f32, tag="ar2_out")
                nc.scalar.dma_start(ar2_in[:], ar2[:].rearrange("p a b -> p (a b)"))
                nc.gpsimd.collective_compute(
                    "AllReduce", add, replica_groups=[list(range(NCORES))],
                    ins=[ar2_in.opt()], outs=[ar2_out.opt()])
                gsb2 = small.tile([P, 3, FO], f32, tag="gsb2")
                nc.scalar.dma_start(gsb2[:].rearrange("p a b -> p (a b)"),
                                    ar2_out[:])

                mu = small.tile([P, FO], f32, tag="mu")
                e2u = small.tile([P, FO], f32, tag="e2u")
                exu = small.tile([P, FO], f32, tag="exu")
                nc.vector.tensor_scalar_mul(mu[:], gsb2[:, 0, :],
                                            256.0 / (N * D))
                nc.vector.tensor_scalar_mul(e2u[:], gsb2[:, 1, :],
                                            1.0 / (N * D))
                nc.vector.tensor_scalar_mul(exu[:], gsb2[:, 2, :], 1.0 / (N * D))
                a1, c1 = affine_from(mu, e2u, gamma1, beta1, "bn1")
                # r = a1*u + c1 + x ; mean_r / E2r
                mean_r = small.tile([P, FO], f32, tag="mean_r")
                e2r = small.tile([P, FO], f32, tag="e2r")
                t8 = small.tile([P, FO], f32, tag="t8")
                nc.vector.tensor_tensor(mean_r[:], a1[:], mu[:], mult)
                nc.vector.tensor_tensor(mean_r[:], mean_r[:], c1[:], add)
                nc.vector.tensor_tensor(mean_r[:], mean_r[:], mx[:], add)
                # E2r = a1*(a1*e2u + 2*(c1*mu + exu)) + c1*(c1 + 2*mx) + e2x
                nc.vector.tensor_tensor(t8[:], c1[:], mu[:], mult)
                nc.vector.tensor_tensor(t8[:], t8[:], exu[:], add)
                nc.vector.tensor_scalar_mul(t8[:], t8[:], 2.0)
                nc.vector.tensor_tensor(e2r[:], a1[:], e2u[:], mult)
                nc.vector.tensor_tensor(e2r[:], e2r[:], t8[:], add)
                nc.vector.tensor_tensor(e2r[:], a1[:], e2r[:], mult)
                nc.vector.tensor_scalar_mul(t8[:], mx[:], 2.0)
                nc.vector.tensor_tensor(t8[:], t8[:], c1[:], add)
                nc.vector.tensor_tensor(t8[:], t8[:], c1[:], mult)
                nc.vector.tensor_tensor(e2r[:], e2r[:], t8[:], add)
                nc.vector.tensor_tensor(e2r[:], e2r[:], e2x[:], add)
                af, cf = affine_from(mean_r, e2r, gammaf, betaf, "bnf")
                # xr = A*u + af*x + Cc ;  A = af*a1, Cc = af*c1 + cf
                Abig = small.tile([P, FO], f32, tag="Abig")
                Cc = small.tile([P, FO], f32, tag="Cc")
                nc.vector.tensor_tensor(Abig[:], af[:], a1[:], mult)
                nc.vector.tensor_tensor(Cc[:], af[:], c1[:], mult)
                nc.vector.tensor_tensor(Cc[:], Cc[:], cf[:], add)
                cnt_u = float(NS * D)

            # vT / zt are dead past AR2: release their SBUF for Phase C pools
            zt_ctx.__exit__(None, None, None)
            vT_zt_ctx.__exit__(None, None, None)

            if STAGE >= 5:
                # ============ PHASE C: xr, t = (Wc+I)@xr + bc ============
                tsb = []
                with tc.tile_pool(name="phc", bufs=NS) as phc, \
                     tc.tile_pool(name="phcs", bufs=4) as phcs, \
                     tc.tile_pool(name="phd", bufs=6) as phd:
                    xrs = []

                    def xr_sample(s):
                        # y = A*u + Cc (ACT) ; xr = af*x + y (fused DVE stt)
                        if s < len(u_pre):
                            us = u_pre[s]
                        else:
                            us = ut_pool.tile([P, FO, D], f16, tag="ut")
                            nc.sync.dma_start(us[:], u_dram[s][:])
                        xhs = xh_pool.tile([P, FO, D], f16, tag="xh")
                        nc.sync.dma_start(xhs[:], xh_io[s])
                        xr = phc.tile([P, FO, D], f16, tag="xr")
                        xrs.append(xr)
                        for fo in range(FO):
                            y = phcs.tile([P, D], f16, tag="y")
                            nc.scalar.activation(y[:], us[:, fo, :],
                                                 Ident, bias=Cc[:, fo:fo + 1],
                                                 scale=Abig[:, fo:fo + 1])
                            nc.vector.scalar_tensor_tensor(
                                out=xr[:, fo, :], in0=xhs[:, fo, :],
                                scalar=af[:, fo:fo + 1], in1=y[:],
                                op0=mult, op1=add)
                        ts = phc.tile([P, FO, D], f16, tag="ts")
                        tsb.append(ts)

                    def conv_sample(wave, s):
                        ocs = range(wave * 4, wave * 4 + 4)
                        pcs = {oc: pmm.tile([P, 512], f32, tag="mm",
                                            name=f"pc{oc}")
                               for oc in ocs}
                        for ic in range(FO):
                            for oc in ocs:
                                nc.tensor.matmul(
                                    pcs[oc][:],
                                    lhsT=WcIT[:, ic, oc * P:(oc + 1) * P],
                                    rhs=xrs[s][:, ic, :],
                                    start=(ic == 0), stop=(ic == FO - 1))
                        for oc in ocs:
                            nc.scalar.activation(tsb[s][:, oc, :],
                                                 pcs[oc][:], Ident,
                                                 bias=bc_col[:, oc:oc + 1],
                                                 scale=1.0)
                            nc.vector.bn_stats(out=tslots[:, oc, s, :],
                                               in_=tsb[s][:, oc, :])

                    def ar3_wave(wave):
                        # aggregate + AllReduce over this wave's 4 channels
                        # lanes: 0 S_t, 1 Q_t
                        oc0 = wave * 4
                        tsl = tslots[:, oc0:oc0 + 4, :, :]
                        ar3 = small.tile([P, 2, 4], f32, name=f"ar3_{wave}")
                        t4 = small.tile([P, 4, NS], f32, name=f"t4_{wave}")
                        t4b = small.tile([P, 4, NS], f32, name=f"t4b_{wave}")
                        tme = tsl[:, :, :, 1]
                        tmo = tsl[:, :, :, 4]
                        nc.vector.tensor_tensor(t4[:], tme, tmo, add)
                        nc.vector.tensor_reduce(out=ar3[:, 0, :], in_=t4[:],
                                                axis=AXX, op=add)
                        nc.vector.tensor_tensor(t4[:], tme, tme, mult)
                        nc.vector.tensor_tensor(t4b[:], tmo, tmo, mult)
                        nc.vector.tensor_tensor(t4[:], t4[:], t4b[:], add)
                        nc.vector.tensor_reduce(out=ar3[:, 1, :], in_=t4[:],
                                                axis=AXX, op=add)
                        nc.vector.tensor_tensor(t4[:], tsl[:, :, :, 2],
                                                tsl[:, :, :, 5], add)
                        nc.vector.tensor_reduce(out=t4b[:, :, 0:1], in_=t4[:],
                                                axis=AXX, op=add)
                        nc.vector.scalar_tensor_tensor(
                            out=ar3[:, 1, :], in0=ar3[:, 1, :], scalar=256.0,
                            in1=t4b[:, :, 0], op0=mult, op1=add)
                        ar3_in = dram.tile([P, 8], f32, tag=f"ar3{wave}_in")
                        ar3_out = dram.tile([P, 8], f32, tag=f"ar3{wave}_out")
                        nc.scalar.dma_start(
                            ar3_in[:], ar3[:].rearrange("p a b -> p (a b)"))
                        nc.gpsimd.collective_compute(
                            "AllReduce", add,
                            replica_groups=[list(range(NCORES))],
                            ins=[ar3_in.opt()], outs=[ar3_out.opt()])
                        return ar3_out

                    def gsb3_read(wave, ar3_out):
                        gsb3 = small.tile([P, 2, 4], f32, name=f"gsb3_{wave}")
                        nc.scalar.dma_start(
                            gsb3[:].rearrange("p a b -> p (a b)"), ar3_out[:])
                        return gsb3

                    def bn_affine_wave(wave, gsb3):
                        ocs = list(range(wave * 4, wave * 4 + 4))
                        mt = small.tile([P, 4], f32, name=f"mt{wave}")
                        e2t = small.tile([P, 4], f32, name=f"e2t{wave}")
                        nc.vector.tensor_scalar_mul(mt[:], gsb3[:, 0, :],
                                                    256.0 / (N * D))
                        nc.vector.tensor_scalar_mul(e2t[:], gsb3[:, 1, :],
                                                    1.0 / (N * D))
                        go4 = small.tile([P, 4], f32, name=f"go{wave}")
                        bo4 = small.tile([P, 4], f32, name=f"bo{wave}")
                        nc.vector.tensor_copy(out=go4[:],
                                              in_=gammao[:, ocs[0]:ocs[0] + 4])
                        nc.vector.tensor_copy(out=bo4[:],
                                              in_=betao[:, ocs[0]:ocs[0] + 4])
                        return affine_from(mt, e2t, go4, bo4, f"bno{wave}",
                                           w=4)

                    def bn_out_wave(wave, ao, co):
                        ocs = list(range(wave * 4, wave * 4 + 4))
                        for s in range(NS):
                            for k, fo in enumerate(ocs):
                                osb = phd.tile([P, D], f32, tag="osb")
                                if fo % 2 == 0:
                                    nc.scalar.activation(
                                        osb[:], tsb[s][:, fo, :], Ident,
                                        bias=co[:, k:k + 1],
                                        scale=ao[:, k:k + 1])
                                else:
                                    nc.vector.tensor_scalar(
                                        out=osb[:], in0=tsb[s][:, fo, :],
                                        scalar1=ao[:, k:k + 1],
                                        scalar2=co[:, k:k + 1],
                                        op0=mult, op1=add)
                                # wave 0 drains on Sync while wave 1 (queued
                                # behind gsb3b on the Activation queue) drains
                                # there — the two 4MB halves run in parallel
                                eng = nc.scalar if wave == 1 else nc.sync
                                eng.dma_start(
                                    out_io[s, fo * P:(fo + 1) * P, :], osb[:])

                    # interleave xr with wave-0 convs so PSUM-releasing bias
                    # ACTs aren't buried behind the xr ACT ops
                    for s in range(NS):
                        xr_sample(s)
                        if DEBUG and s == 0:
                            with tc.tile_pool(name="dbgx", bufs=1) as dbgx:
                                for fc in range(FO):
                                    st = dbgx.tile([P, D], f32, tag="st")
                                    nc.vector.tensor_copy(
                                        out=st[:], in_=xrs[0][:, fc, :])
                                    nc.sync.dma_start(dbg_xr[:, fc, :], st[:])
                        conv_sample(0, s)
                    ar3a = ar3_wave(0)
                    g3a = gsb3_read(0, ar3a)
                    conv_sample(1, 0)
                    conv_sample(1, 1)
                    # wave-0 affine: its DVE ops queue behind samples 0-1's
                    # wave-1 stats only, so they run mid-wave-1
                    ao0, co0 = bn_affine_wave(0, g3a)
                    conv_sample(1, 2)
                    conv_sample(1, 3)
                    ar3b = ar3_wave(1)   # trigger ASAP after s3 stats
                    g3b = gsb3_read(1, ar3b)
                    # wave-0 applies + output DMAs run during AR3b
                    bn_out_wave(0, ao0, co0)
                    ao1, co1 = bn_affine_wave(1, g3b)
                    bn_out_wave(1, ao1, co1)

    nc.compile()
    return nc


def _get_nc(has_bias=False):
    key = ("nc", has_bias)
    if key not in _CACHE:
        _CACHE[key] = _build(has_bias)
    return _CACHE[key]


def make_in_maps(inputs):
    """Host-side prep: shard x over cores, pre-transpose/cast weights."""
    x = np.ascontiguousarray(inputs["x"], dtype=np.float32)
    W0 = np.asarray(inputs["W0"], dtype=np.float32)
    W1 = np.asarray(inputs["W1"], dtype=np.float32)
    Wc = np.asarray(inputs["Wc"], dtype=np.float32)
    b0 = np.asarray(inputs["b0"], dtype=np.float32)
    b1 = np.asarray(inputs["b1"], dtype=np.float32)
    bc = np.asarray(inputs["bc"], dtype=np.float32)
    has_bias = bool(np.any(b0) or np.any(b1))

    # W0T[di, do, h] = W0[h, do*128+di]
    W0T = np.ascontiguousarray(
        W0.reshape(H, DO, P).transpose(2, 1, 0).astype(np.float16))
    # M[e, d] = (W1 @ W0)[d, e] -> z_tilde = x @ M ; MT[di, do, d]
    M = (W1 @ W0).T
    MT = np.ascontiguousarray(
        M.reshape(DO, P, D).transpose(1, 0, 2).astype(np.float16))
    # s1r[p, d] = sum_h W1[d, h], broadcast over partitions
    s1 = W1.sum(axis=1)
    s1r = np.ascontiguousarray(
        np.broadcast_to(s1[None, :], (P, D)).astype(np.float16))
    # WcIT[ii, io, o] = (Wc+I)[o, io*128+ii]
    WcI = Wc + np.eye(F, dtype=np.float32)
    WcIT = np.ascontiguousarray(
        WcI.reshape(F, FO, P).transpose(2, 1, 0).astype(np.float16))

    # prm [P, 9, FO]: g0, be0, g1, be1, gf, bf, go, bo, bc
    prm = np.stack([np.asarray(inputs[k], dtype=np.float32)
                    .reshape(FO, P).T for k in
                    ["g0", "be0", "g1", "be1", "gf", "bf", "go", "bo"]] +
                   [bc.reshape(FO, P).T], axis=1)
    prm = np.ascontiguousarray(prm)  # [P, 9, FO]

    shared = {"W0T": W0T, "MT": MT, "s1r": s1r, "WcIT": WcIT, "prm": prm}
    if has_bias:
        shared["b0r"] = np.ascontiguousarray(b0.reshape(1, H))
        shared["b1r"] = np.ascontiguousarray(b1.reshape(1, D))

    in_maps = []
    for c in range(NCORES):
        xs = x[c * NS:(c + 1) * NS]  # [NS, F, D]
        # xh[s, fi, fo, d] = x[s, fo*128+fi, d]
        xh = np.ascontiguousarray(
            xs.reshape(NS, FO, P, D).transpose(0, 2, 1, 3).astype(np.float16))
        # xT[s, di, do, f] = x[s, f, do*128+di]
        xT = np.ascontiguousarray(
            xs.reshape(NS, F, DO, P).transpose(0, 3, 2, 1).astype(np.float16))
        m = {"xh": xh, "xT": xT}
        m.update(shared)
        in_maps.append(m)
    return in_maps, has_bias


def kernel(**inputs) -> np.ndarray:
    from concourse import bass_utils

    in_maps, has_bias = make_in_maps(inputs)
    nc = _get_nc(has_bias)
    res = bass_utils.run_bass_kernel_spmd(
        nc, in_maps, core_ids=list(range(NCORES)), trace=False)
    out = np.concatenate([res.results[c]["out"] for c in range(NCORES)],
                         axis=0)
    return out.astype(np.float32)


# revision 55
# speedup vs baseline: 1.0208x; 1.0208x over previous
"""Trainium2 Bass kernel for nn_Correlation_Block (N=32, F=1024, D=512, H=1024).

Data-parallel over batch N across 8 NeuronCores (4 samples each).
BatchNorm batch statistics combined across cores with tiny AllReduces
(plus two dummy warmup AllReduces at kernel start to ramp the CC cores).

Host-side (free, not in HW exec time):
  - x passed twice as fp16: xh [fi,fo,d] and xT [di,do,f] (no device transposes)
  - weights passed fp16 pre-transposed: W0T [di,do,h],
    WcIT [ii,io,o] with (Wc + I) folded so t = conv(xr)+xr is one matmul chain
  - MT [di,do,d] = (W0^T W1^T) reshaped: z-tilde = x @ M collapses the
    K=1024 contraction of z = v1 @ W1^T to K=512 (half the PE work); the
    BN0 affine is restored with z = a0*zt + c0 x s1row (s1 = row sums of W1)
  - BN gamma/beta pre-tiled [P, FO]

Device-side per sample:
  v0 = x @ W0^T            (64 MMs, K=512) + bn stats
  zt = x @ M               (32 MMs, K=512)
  v0T                      (64 PE transposes, pre-affine, in Phase A ->
                            no v0 DRAM spill; they also fill the AR1 window)
  [AR1] -> a0/c0; aB/cB row-broadcast tiles built via PE transpose + DMA
  v1T = aB*v0T + cB        (DVE, broadcast affine along free dim)
  w  = softsign(v1 @ v1^T) (96 MMs using symmetry; 4 mirrored tiles)
  z  = a0*zt + c0*s1row    (DVE fixup in place)
  u  = w @ z               (64 MMs, sw used as lhsT via symmetry); u spills
                           to DRAM (SBUF is too tight to hold u + v1T + zt)
  [AR2] -> merged BN1+feed_norm affines
  xr = A*u + af*x + Cc     (ACT + fused DVE stt)
  t  = (Wc+I) @ xr (+bc)   (64 MMs in two 4-channel waves across samples;
                            each wave's stats AllReduce + BN apply + output
                            DMA overlap the other wave's matmuls)
"""

import numpy as np

N, F, D = 32, 1024, 512
H = 1024
NCORES = 8
NS = N // NCORES          # samples per core
EPS = 1e-5
P = 128
FO = F // P               # 8 f-chunks
DO = D // P               # 4 d-chunks
HO = H // P               # 8 h-chunks
HH = H // 512             # 2 (512-wide halves of H)

_CACHE = {}

import os
STAGE = int(os.environ.get("BASS_STAGE", "99"))
DEBUG = int(os.environ.get("BASS_DEBUG", "0"))


def _build(has_bias):
    import concourse.bass as bass
    import concourse.tile as tile
    from concourse import bacc, mybir
    from concourse.masks import make_identity

    f32 = mybir.dt.float32
    f16 = mybir.dt.float16

    nc = bacc.Bacc("TRN2", target_bir_lowering=False, debug=False,
                   num_devices=NCORES)

    # ---- I/O ----
    xh_io = nc.dram_tensor("xh", [NS, P, FO, D], f16, kind="ExternalInput").ap()
    xT_io = nc.dram_tensor("xT", [NS, P, DO, F], f16, kind="ExternalInput").ap()
    W0T_io = nc.dram_tensor("W0T", [P, DO, H], f16, kind="ExternalInput").ap()
    MT_io = nc.dram_tensor("MT", [P, DO, D], f16, kind="ExternalInput").ap()
    s1r_io = nc.dram_tensor("s1r", [P, D], f16, kind="ExternalInput").ap()
    WcIT_io = nc.dram_tensor("WcIT", [P, FO, F], f16, kind="ExternalInput").ap()
    prm_io = nc.dram_tensor("prm", [P, 9, FO], f32, kind="ExternalInput").ap()
    if has_bias:
        b0r_io = nc.dram_tensor("b0r", [1, H], f32, kind="ExternalInput").ap()
        b1r_io = nc.dram_tensor("b1r", [1, D], f32, kind="ExternalInput").ap()
    out_io = nc.dram_tensor("out", [NS, F, D], f32, kind="ExternalOutput").ap()
    if DEBUG:
        dbg_ac = nc.dram_tensor("dbg_ac", [P, 4, F], f32,
                                kind="ExternalOutput").ap()
        dbg_vT = nc.dram_tensor("dbg_vT", [P, HO, F], f32,
                                kind="ExternalOutput").ap()
        dbg_zt = nc.dram_tensor("dbg_zt", [P, FO, D], f32,
                                kind="ExternalOutput").ap()
        dbg_sw = nc.dram_tensor("dbg_sw", [P, FO, F], f32,
                                kind="ExternalOutput").ap()
        dbg_u = nc.dram_tensor("dbg_u", [P, FO, D], f32,
                               kind="ExternalOutput").ap()
        dbg_xr = nc.dram_tensor("dbg_xr", [P, FO, D], f32,
                                kind="ExternalOutput").ap()

    add = mybir.AluOpType.add
    sub = mybir.AluOpType.subtract
    mult = mybir.AluOpType.mult
    Ident = mybir.ActivationFunctionType.Identity
    Copy = mybir.ActivationFunctionType.Copy
    Sqrt = mybir.ActivationFunctionType.Sqrt
    Abs = mybir.ActivationFunctionType.Abs
    AXX = mybir.AxisListType.X

    with tile.TileContext(nc) as tc:
        with tc.tile_pool(name="persist", bufs=1) as persist, \
             tc.tile_pool(name="xh", bufs=2) as xh_pool, \
             tc.tile_pool(name="ut", bufs=2) as ut_pool, \
             tc.tile_pool(name="small", bufs=1) as small, \
             tc.tile_pool(name="pmm", bufs=6, space="PSUM") as pmm, \
             tc.tile_pool(name="ptr", bufs=2, space="PSUM") as ptr, \
             tc.tile_pool(name="dram", bufs=1, space="DRAM") as dram:

            # ---- three chained dummy AllReduces (64KB): absorb CC-core
            # boot (~50us) and keep the CC busy/warm until AR1 ----
            ar0 = small.tile([P, 128], f32)
            nc.vector.memset(ar0[:], 1.0)
            ar0_in = dram.tile([P, 128], f32, tag="ar0_in")
            ar0_out = dram.tile([P, 128], f32, tag="ar0_out")
            ar0b_out = dram.tile([P, 128], f32, tag="ar0b_out")
            ar0c_out = dram.tile([P, 128], f32, tag="ar0c_out")
            nc.scalar.dma_start(ar0_in[:], ar0[:])
            nc.gpsimd.collective_compute(
                "AllReduce", add, replica_groups=[list(range(NCORES))],
                ins=[ar0_in.opt()], outs=[ar0_out.opt()])
            nc.gpsimd.collective_compute(
                "AllReduce", add, replica_groups=[list(range(NCORES))],
                ins=[ar0_out.opt()], outs=[ar0b_out.opt()])
            nc.gpsimd.collective_compute(
                "AllReduce", add, replica_groups=[list(range(NCORES))],
                ins=[ar0b_out.opt()], outs=[ar0c_out.opt()])
            # read-back deferred (a waiting DMA would block its queue)

            ident = persist.tile([P, P], f16)
            make_identity(nc, ident[:])

            eps_col = persist.tile([P, 1], f32)
            nc.vector.memset(eps_col[:], EPS)

            # ---- weights (host-prepared, straight DMA) ----
            # Weights ride the Activation HWDGE queue, x loads ride Sync.
            W0T = persist.tile([P, DO, H], f16)
            MT = persist.tile([P, DO, D], f16)
            s1r = persist.tile([P, D], f16)
            WcIT = persist.tile([P, FO, F], f16)
            prm = persist.tile([P, 9, FO], f32)
            gamma0 = prm[:, 0, :]
            beta0 = prm[:, 1, :]
            gamma1 = prm[:, 2, :]
            beta1 = prm[:, 3, :]
            gammaf = prm[:, 4, :]
            betaf = prm[:, 5, :]
            gammao = prm[:, 6, :]
            betao = prm[:, 7, :]
            bc_col = prm[:, 8, :]
            # per-dc split so the first matmul only waits for its own slice
            for dc in range(DO):
                nc.scalar.dma_start(W0T[:, dc, :], W0T_io[:, dc, :])
            nc.scalar.dma_start(MT[:], MT_io)

            if has_bias:
                xone = persist.tile([P, P], f16)
                nc.vector.memset(xone[:], 0.0)
                nc.vector.memset(xone[0:1, :], 1.0)
                W0b = persist.tile([P, H], f16)
                W1b = persist.tile([P, D], f16)
                nc.vector.memset(W0b[:], 0.0)
                nc.vector.memset(W1b[:], 0.0)
                with tc.tile_pool(name="btmp", bufs=2) as btmp:
                    t = btmp.tile([1, H], f32, tag="b")
                    nc.sync.dma_start(t[:], b0r_io)
                    nc.vector.tensor_copy(out=W0b[0:1, :], in_=t[:])
                    t = btmp.tile([1, D], f32, tag="b")
                    nc.sync.dma_start(t[:], b1r_io)
                    nc.vector.tensor_copy(out=W1b[0:1, :], in_=t[:])

            # stats slot tiles
            vslots = small.tile([P, FO, HH * NS, 6], f32)
            xslots = small.tile([P, FO, NS, 6], f32)
            uslots = small.tile([P, FO, NS, 6], f32)
            xuslots = small.tile([P, FO * NS], f32)
            tslots = small.tile([P, FO, NS, 6], f32)

            vT = []       # per-sample v0T (pre-affine) [P, HO, F]
            ztl = []      # per-sample z-tilde [P, FO, D]
            u_dram = []

            vT_zt_ctx = tc.tile_pool(name="vT", bufs=1)
            vT_pool = vT_zt_ctx.__enter__()
            zt_ctx = tc.tile_pool(name="zt", bufs=1)
            zt_pool = zt_ctx.__enter__()

            if STAGE >= 1:
                # ============ PHASE A ============
                # per sample: v0 = x@W0T (+stats), transposes, zt = x@M
                with tc.tile_pool(name="xT", bufs=2) as xT_pool, \
                     tc.tile_pool(name="v0sb", bufs=1) as v0_pool:
                    for s in range(NS):
                        xTs = xT_pool.tile([P, DO, F], f16, tag="xT")
                        if s == 0:
                            for dc in range(DO):
                                nc.sync.dma_start(xTs[:, dc, :],
                                                  xT_io[s, :, dc, :])
                        else:
                            nc.sync.dma_start(xTs[:], xT_io[s])
                        xhs = xh_pool.tile([P, FO, D], f16, tag="xh")
                        nc.sync.dma_start(xhs[:], xh_io[s])
                        if s == 0:
                            nc.scalar.dma_start(s1r[:], s1r_io)
                        if s == 1:
                            nc.scalar.dma_start(WcIT[:], WcIT_io)
                            nc.scalar.dma_start(prm[:], prm_io)
                        # xh stats first (they only need the DMA; keep them
                        # off the post-matmul DVE critical path)
                        for fo in range(FO):
                            nc.vector.bn_stats(out=xslots[:, fo, s, :],
                                               in_=xhs[:, fo, :])
                        vTs = vT_pool.tile([P, HO, F], f16, name=f"vT{s}")
                        vT.append(vTs)
                        # hh-split: compute the 512-wide h-half for all fc,
                        # transpose it, then the other half (v0sb is only one
                        # half-H wide to save SBUF)
                        for hh in range(HH):
                            v0sb = v0_pool.tile([P, FO, 512], f16, tag="v0")
                            for fc in range(FO):
                                pv = pmm.tile([P, 512], f32, tag="mm")
                                for dc in range(DO):
                                    nc.tensor.matmul(
                                        pv[:],
                                        lhsT=xTs[:, dc, fc * P:(fc + 1) * P],
                                        rhs=W0T[:, dc, hh * 512:(hh + 1) * 512],
                                        start=(dc == 0),
                                        stop=(dc == DO - 1 and not has_bias))
                                if has_bias:
                                    nc.tensor.matmul(
                                        pv[:], lhsT=xone[:],
                                        rhs=W0b[:, hh * 512:(hh + 1) * 512],
                                        start=False, stop=True)
                                nc.vector.bn_stats(
                                    out=vslots[:, fc, hh * NS + s, :], in_=pv[:])
                                nc.scalar.activation(
                                    v0sb[:, fc, :], pv[:], Copy)
                            # v0T transposes (pre-affine); the last sample's
                            # run inside the AR1 window
                            for hl in range(4):
                                ho = hh * 4 + hl
                                pt = ptr.tile([P, F], f16, tag="tr")
                                for fc in range(FO):
                                    nc.tensor.transpose(
                                        pt[:, fc * P:(fc + 1) * P],
                                        v0sb[:, fc, hl * P:(hl + 1) * P],
                                        ident[:])
                                nc.scalar.activation(vTs[:, ho, :], pt[:], Copy)
                        # zt = x @ M  (K=512; half the work of v1@W1T)
                        zts = zt_pool.tile([P, FO, D], f16, name=f"zt{s}")
                        ztl.append(zts)
                        for fc in range(FO):
                            pz = pmm.tile([P, 512], f32, tag="mm")
                            for dc in range(DO):
                                nc.tensor.matmul(
                                    pz[:],
                                    lhsT=xTs[:, dc, fc * P:(fc + 1) * P],
                                    rhs=MT[:, dc, :],
                                    start=(dc == 0), stop=(dc == DO - 1))
                            nc.vector.tensor_copy(out=zts[:, fc, :], in_=pz[:])

            if STAGE >= 2:
                # ---- aggregate + AllReduce 1 ----
                # Wide-op aggregation from the bn_stats 6-tuples (lanes:
                # cnt_e, mean_e, M2_e, cnt_o, mean_o, M2_o; 256 elements per
                # parity).  AR lanes: 0 S_v, 1 Q_v, 2 S_x, 3 Q_x where
                # S = sum of means, Q = 256*sum(mean^2) + sum(M2); the 256x
                # count factors fold into the post-AR scaling.
                ar1 = small.tile([P, 4, FO], f32, tag="ar1")
                NSL = HH * NS
                tv = small.tile([P, FO, NSL], f32, tag="tv")
                tx = small.tile([P, FO, NS], f32, tag="tx")

                def slot_reduce(slots, t, lane_off, nsl):
                    me = slots[:, :, :, 1]
                    mo = slots[:, :, :, 4]
                    t2 = small.tile([P, FO, nsl], f32, name=f"t2s_{lane_off}")
                    nc.vector.tensor_tensor(t[:], me, mo, add)
                    nc.vector.tensor_reduce(
                        out=ar1[:, lane_off + 0, :], in_=t[:], axis=AXX, op=add)
                    nc.vector.tensor_tensor(t[:], me, me, mult)
                    nc.vector.tensor_tensor(t2[:], mo, mo, mult)
                    nc.vector.tensor_tensor(t[:], t[:], t2[:], add)
                    nc.vector.tensor_reduce(
                        out=ar1[:, lane_off + 1, :], in_=t[:], axis=AXX, op=add)
                    nc.vector.tensor_tensor(t[:], slots[:, :, :, 2],
                                            slots[:, :, :, 5], add)
                    nc.vector.tensor_reduce(
                        out=t2[:, :, 0:1], in_=t[:], axis=AXX, op=add)
                    nc.vector.scalar_tensor_tensor(
                        out=ar1[:, lane_off + 1, :],
                        in0=ar1[:, lane_off + 1, :], scalar=256.0,
                        in1=t2[:, :, 0], op0=mult, op1=add)

                slot_reduce(vslots, tv, 0, NSL)
                slot_reduce(xslots, tx, 2, NS)

                ar1_in = dram.tile([P, 4 * FO], f32, tag="ar1_in")
                ar1_out = dram.tile([P, 4 * FO], f32, tag="ar1_out")
                nc.scalar.dma_start(ar1_in[:], ar1[:].rearrange("p a b -> p (a b)"))
                nc.gpsimd.collective_compute(
                    "AllReduce", add, replica_groups=[list(range(NCORES))],
                    ins=[ar1_in.opt()], outs=[ar1_out.opt()])
                gsb1 = small.tile([P, 4, FO], f32, tag="gsb1")
                nc.scalar.dma_start(gsb1[:].rearrange("p a b -> p (a b)"),
                                    ar1_out[:])
                # deferred dummy-AR read-back (long done; software queue)
                ar0_back = small.tile([P, 128], f32, tag="ar0b")
                nc.gpsimd.dma_start(ar0_back[:], ar0c_out[:])

                def affine_from(mean_t, e2_t, gamma_t, beta_t, nm, w=FO):
                    """returns (a, c) tiles [P, w]"""
                    var_t = small.tile([P, w], f32, name=f"var_{nm}")
                    t2 = small.tile([P, w], f32, name=f"t2_{nm}")
                    nc.vector.tensor_tensor(t2[:], mean_t[:], mean_t[:], mult)
                    nc.vector.tensor_tensor(var_t[:], e2_t[:], t2[:], sub)
                    sd = small.tile([P, w], f32, name=f"sd_{nm}")
                    nc.scalar.activation(sd[:], var_t[:], Sqrt,
                                         bias=eps_col[:], scale=1.0)
                    nc.vector.reciprocal(sd[:], sd[:])
                    a_t = small.tile([P, w], f32, name=f"a_{nm}")
                    c_t = small.tile([P, w], f32, name=f"c_{nm}")
                    nc.vector.tensor_tensor(a_t[:], gamma_t[:], sd[:], mult)
                    nc.vector.tensor_tensor(t2[:], mean_t[:], a_t[:], mult)
                    nc.vector.tensor_tensor(c_t[:], beta_t[:], t2[:], sub)
                    return a_t, c_t

                # m = 256*S/(N*H); E2 = Q/(N*H)
                m0 = small.tile([P, FO], f32, tag="m0")
                e20 = small.tile([P, FO], f32, tag="e20")
                nc.vector.tensor_scalar_mul(m0[:], gsb1[:, 0, :],
                                            256.0 / (N * H))
                nc.vector.tensor_scalar_mul(e20[:], gsb1[:, 1, :],
                                            1.0 / (N * H))
                a0, c0 = affine_from(m0, e20, gamma0, beta0, "bn0")
                mx = small.tile([P, FO], f32, tag="mx")
                e2x = small.tile([P, FO], f32, tag="e2x")
                nc.vector.tensor_scalar_mul(mx[:], gsb1[:, 2, :],
                                            256.0 / (N * D))
                nc.vector.tensor_scalar_mul(e2x[:], gsb1[:, 3, :],
                                            1.0 / (N * D))

                # ---- aB/cB: broadcast a0/c0 along partitions as [P, F]
                # rows (f = fo*128+fi on the free axis).  Per fo-block:
                # replicate the per-partition coefficient column along free
                # (tensor_scalar by ones), then PE-transpose the block. ----
                ones128 = small.tile([P, P], f16, tag="ones128")
                nc.vector.memset(ones128[:], 1.0)
                acB = persist.tile([P, 2, F], f16)
                for lane, coef in enumerate([a0, c0]):
                    rep = small.tile([P, F], f16, name=f"rep{lane}")
                    for fo in range(FO):
                        nc.vector.tensor_scalar(
                            out=rep[:, fo * P:(fo + 1) * P], in0=ones128[:],
                            scalar1=coef[:, fo:fo + 1], scalar2=None, op0=mult)
                    pb = ptr.tile([P, F], f16, tag="tr")
                    for fo in range(FO):
                        nc.tensor.transpose(
                            pb[:, fo * P:(fo + 1) * P],
                            rep[:, fo * P:(fo + 1) * P], ident[:])
                    nc.scalar.activation(acB[:, lane, :], pb[:], Copy)
                aB = acB[:, 0, :]
                cB = acB[:, 1, :]
                if DEBUG:
                    with tc.tile_pool(name="dbgp", bufs=1) as dbgp:
                        st = dbgp.tile([P, F], f32, tag="st")
                        nc.vector.tensor_copy(out=st[:], in_=aB)
                        nc.sync.dma_start(dbg_ac[:, 0, :], st[:])
                        st = dbgp.tile([P, F], f32, tag="st")
                        nc.vector.tensor_copy(out=st[:], in_=cB)
                        nc.sync.dma_start(dbg_ac[:, 1, :], st[:])
                        st = dbgp.tile([P, F], f32, tag="st")
                        nc.vector.memset(st[:], 0.0)
                        nc.vector.tensor_copy(out=st[:, 0:FO], in_=a0[:])
                        nc.vector.tensor_copy(out=st[:, FO:2 * FO], in_=c0[:])
                        nc.vector.tensor_copy(out=st[:, 2 * FO:3 * FO],
                                              in_=m0[:])
                        nc.vector.tensor_copy(out=st[:, 3 * FO:4 * FO],
                                              in_=e20[:])
                        nc.sync.dma_start(dbg_ac[:, 2, :], st[:])

            if STAGE >= 3:
                # ============ PHASE B: w, z-fixup, u ============
                with tc.tile_pool(name="phb", bufs=1) as phb, \
                     tc.tile_pool(name="phbs", bufs=2) as phbs:
                    xh_b = {}

                    def prep_vt(s):
                        # v1T = aB*v0T + cB (broadcast affine, in place);
                        # emitted one sample ahead so the next w-matmuls
                        # never wait on DVE.  xh reload for the xu term.
                        for ho in range(HO):
                            nc.vector.tensor_tensor(
                                vT[s][:, ho, :], vT[s][:, ho, :], aB, mult)
                            nc.vector.tensor_tensor(
                                vT[s][:, ho, :], vT[s][:, ho, :], cB, add)
                        xhs = xh_pool.tile([P, FO, D], f16, tag="xh")
                        nc.sync.dma_start(xhs[:], xh_io[s])
                        xh_b[s] = xhs

                    def prep_z(s):
                        # z = a0*zt + c0*s1row (in place); only needed by
                        # sample s's u-matmuls, so this can run late
                        for fc in range(FO):
                            nc.vector.tensor_scalar(
                                out=ztl[s][:, fc, :], in0=ztl[s][:, fc, :],
                                scalar1=a0[:, fc:fc + 1], scalar2=None,
                                op0=mult)
                            nc.vector.scalar_tensor_tensor(
                                out=ztl[s][:, fc, :], in0=s1r[:],
                                scalar=c0[:, fc:fc + 1], in1=ztl[s][:, fc, :],
                                op0=mult, op1=add)

                    prep_vt(0)
                    prep_z(0)
                    for s in range(NS):
                        xhs = xh_b[s]
                        if DEBUG and s == 0:
                            with tc.tile_pool(name="dbgb", bufs=1) as dbgb:
                                for ho in range(HO):
                                    st = dbgb.tile([P, F], f32, tag="st")
                                    nc.vector.tensor_copy(
                                        out=st[:], in_=vT[s][:, ho, :])
                                    nc.sync.dma_start(dbg_vT[:, ho, :], st[:])
                                for fc in range(FO):
                                    st = dbgb.tile([P, F], f32, tag="st")
                                    nc.vector.tensor_copy(
                                        out=st[:, 0:D], in_=ztl[s][:, fc, :])
                                    nc.sync.dma_start(dbg_zt[:, fc, :],
                                                      st[:, 0:D])

                        # w = softsign(v1 @ v1^T), exploiting symmetry
                        swsb = phb.tile([P, FO, F], f16, tag="sw")

                        def w_tile(fc, gg):
                            pw = pmm.tile([P, 512], f32, tag="mm")
                            for ho in range(HO):
                                nc.tensor.matmul(
                                    pw[:],
                                    lhsT=vT[s][:, ho, fc * P:(fc + 1) * P],
                                    rhs=vT[s][:, ho, gg * 512:(gg + 1) * 512],
                                    start=(ho == 0), stop=(ho == HO - 1))
                            absw = phbs.tile([P, 512], f32, tag="absw")
                            nc.scalar.activation(absw[:], pw[:], Abs)
                            nc.scalar.add(absw[:], absw[:], 1.0)
                            rcp = phbs.tile([P, 512], f32, tag="rcp")
                            nc.vector.reciprocal_approx_fast(rcp[:], absw[:])
                            nc.vector.tensor_tensor(
                                swsb[:, fc, gg * 512:(gg + 1) * 512],
                                pw[:], rcp[:], mult)

                        for fc in range(4):
                            for gg in range(HH):
                                w_tile(fc, gg)
                            if fc == 1 and s + 1 < NS:
                                # next sample's vT affine: DVE has slack here
                                # (softsign only), and the ops land well
                                # before sample s+1's w-matmuls need them
                                prep_vt(s + 1)
                        # mirrors: swsb[:, fc4, rc*P:+P] =
                        #   transpose(swsb[:, rc, fc4*P:+P]) for rc 0..3
                        for fc4 in range(4, 8):
                            pt2 = ptr.tile([P, 512], f16, tag="tr")
                            for rc in range(4):
                                nc.tensor.transpose(
                                    pt2[:, rc * P:(rc + 1) * P],
                                    swsb[:, rc, fc4 * P:(fc4 + 1) * P],
                                    ident[:])
                            nc.scalar.activation(
                                swsb[:, fc4, 0:512], pt2[:], Copy)
                        for fc in range(4, 8):
                            w_tile(fc, 1)

                        # u = w @ z  (sw as lhsT via symmetry)
                        us = ut_pool.tile([P, FO, D], f16, tag="ut")
                        for fc in range(FO):
                            if fc == 5 and s + 1 < NS:
                                # next sample's z fixup: DVE catches up with
                                # this sample's u stats by mid-loop
                                prep_z(s + 1)
                            pu = pmm.tile([P, 512], f32, tag="mm")
                            for gc in range(FO):
                                nc.tensor.matmul(
                                    pu[:],
                                    lhsT=swsb[:, gc, fc * P:(fc + 1) * P],
                                    rhs=ztl[s][:, gc, :],
                                    start=(gc == 0),
                                    stop=(gc == FO - 1 and not has_bias))
                            if has_bias:
                                nc.tensor.matmul(
                                    pu[:], lhsT=xone[:], rhs=W1b[:],
                                    start=False, stop=True)
                            nc.vector.bn_stats(out=uslots[:, fc, s, :], in_=pu[:])
                            junk = phbs.tile([P, 512], f32, tag="junk")
                            nc.vector.tensor_tensor(
                                junk[:], pu[:], xhs[:, fc, :], mult)
                            nc.vector.tensor_reduce(
                                out=xuslots[:, fc * NS + s:fc * NS + s + 1],
                                in_=junk[:], axis=AXX, op=add)
                            nc.scalar.activation(us[:, fc, :], pu[:], Copy)
                        ud = dram.tile([P, FO, D], f16, tag=f"ud{s}")
                        u_dram.append(ud)
                        nc.sync.dma_start(ud[:], us[:])
                        if DEBUG and s == 0:
                            with tc.tile_pool(name="dbgu", bufs=1) as dbgu:
                                for fc in range(FO):
                                    st = dbgu.tile([P, F], f32, tag="st")
                                    nc.vector.tensor_copy(
                                        out=st[:], in_=swsb[:, fc, :])
                                    nc.sync.dma_start(dbg_sw[:, fc, :], st[:])
                                    st = dbgu.tile([P, F], f32, tag="st")
                                    nc.vector.tensor_copy(
                                        out=st[:, 0:D], in_=us[:, fc, :])
                                    nc.sync.dma_start(dbg_u[:, fc, :],
                                                      st[:, 0:D])

            # pre-issue u reloads for the first two Phase-C samples so they
            # run during AR2
            u_pre = []
            if STAGE >= 5:
                for s in range(min(2, NS)):
                    upb = ut_pool.tile([P, FO, D], f16, tag="ut",
                                       name=f"upre{s}")
                    nc.sync.dma_start(upb[:], u_dram[s][:])
                    u_pre.append(upb)

            if STAGE >= 4:
                # ---- aggregate + AllReduce 2 ----
                # lanes: 0 S_u, 1 Q_u, 2 sum(x*u)
                ar2 = small.tile([P, 3, FO], f32, tag="ar2")
                tu = small.tile([P, FO, NS], f32, tag="tu")
                tu2 = small.tile([P, FO, NS], f32, tag="tu2")
                ume = uslots[:, :, :, 1]
                umo = uslots[:, :, :, 4]
                nc.vector.tensor_tensor(tu[:], ume, umo, add)
                nc.vector.tensor_reduce(out=ar2[:, 0, :], in_=tu[:],
                                        axis=AXX, op=add)
                nc.vector.tensor_tensor(tu[:], ume, ume, mult)
                nc.vector.tensor_tensor(tu2[:], umo, umo, mult)
                nc.vector.tensor_tensor(tu[:], tu[:], tu2[:], add)
                nc.vector.tensor_reduce(out=ar2[:, 1, :], in_=tu[:],
                                        axis=AXX, op=add)
                nc.vector.tensor_tensor(tu[:], uslots[:, :, :, 2],
                                        uslots[:, :, :, 5], add)
                nc.vector.tensor_reduce(out=tu2[:, :, 0:1], in_=tu[:],
                                        axis=AXX, op=add)
                nc.vector.scalar_tensor_tensor(
                    out=ar2[:, 1, :], in0=ar2[:, 1, :], scalar=256.0,
                    in1=tu2[:, :, 0], op0=mult, op1=add)
                nc.vector.tensor_reduce(
                    out=ar2[:, 2, :],
                    in_=xuslots[:].rearrange("p (fo s) -> p fo s", s=NS),
                    axis=AXX, op=add)

                ar2_in = dram.tile([P, 3 * FO], f32, tag="ar2_in")
                ar2_out = dram.tile([P, 3 * FO], f32, tag="ar2_out")
                nc.scalar.dma_start(ar2_in[:], ar2[:].rearrange("p a b -> p (a b)"))
                nc.gpsimd.collective_compute(
                    "AllReduce", add, replica_groups=[list(range(NCORES))],
                    ins=[ar2_in.opt()], outs=[ar2_out.opt()])
                gsb2 = small.tile([P, 3, FO], f32, tag="gsb2")
                nc.scalar.dma_start(gsb2[:].rearrange("p a b -> p (a b)"),
                                    ar2_out[:])

                mu = small.tile([P, FO], f32, tag="mu")
                e2u = small.tile([P, FO], f32, tag="e2u")
                exu = small.tile([P, FO], f32, tag="exu")
                nc.vector.tensor_scalar_mul(mu[:], gsb2[:, 0, :],
                                            256.0 / (N * D))
                nc.vector.tensor_scalar_mul(e2u[:], gsb2[:, 1, :],
                                            1.0 / (N * D))
                nc.vector.tensor_scalar_mul(exu[:], gsb2[:, 2, :], 1.0 / (N * D))
                a1, c1 = affine_from(mu, e2u, gamma1, beta1, "bn1")
                # r = a1*u + c1 + x ; mean_r / E2r
                mean_r = small.tile([P, FO], f32, tag="mean_r")
                e2r = small.tile([P, FO], f32, tag="e2r")
                t8 = small.tile([P, FO], f32, tag="t8")
                nc.vector.tensor_tensor(mean_r[:], a1[:], mu[:], mult)
                nc.vector.tensor_tensor(mean_r[:], mean_r[:], c1[:], add)
                nc.vector.tensor_tensor(mean_r[:], mean_r[:], mx[:], add)
                # E2r = a1*(a1*e2u + 2*(c1*mu + exu)) + c1*(c1 + 2*mx) + e2x
                nc.vector.tensor_tensor(t8[:], c1[:], mu[:], mult)
                nc.vector.tensor_tensor(t8[:], t8[:], exu[:], add)
                nc.vector.tensor_scalar_mul(t8[:], t8[:], 2.0)
                nc.vector.tensor_tensor(e2r[:], a1[:], e2u[:], mult)
                nc.vector.tensor_tensor(e2r[:], e2r[:], t8[:], add)
                nc.vector.tensor_tensor(e2r[:], a1[:], e2r[:], mult)
                nc.vector.tensor_scalar_mul(t8[:], mx[:], 2.0)
                nc.vector.tensor_tensor(t8[:], t8[:], c1[:], add)
                nc.vector.tensor_tensor(t8[:], t8[:], c1[:], mult)
                nc.vector.tensor_tensor(e2r[:], e2r[:], t8[:], add)
                nc.vector.tensor_tensor(e2r[:], e2r[:], e2x[:], add)
                af, cf = affine_from(mean_r, e2r, gammaf, betaf, "bnf")
                # xr = A*u + af*x + Cc ;  A = af*a1, Cc = af*c1 + cf
                Abig = small.tile([P, FO], f32, tag="Abig")
                Cc = small.tile([P, FO], f32, tag="Cc")
                nc.vector.tensor_tensor(Abig[:], af[:], a1[:], mult)
                nc.vector.tensor_tensor(Cc[:], af[:], c1[:], mult)
                nc.vector.tensor_tensor(Cc[:], Cc[:], cf[:], add)
                cnt_u = float(NS * D)

            # vT / zt are dead past AR2: release their SBUF for Phase C pools
            zt_ctx.__exit__(None, None, None)
            vT_zt_ctx.__exit__(None, None, None)

            if STAGE >= 5:
                # ============ PHASE C: xr, t = (Wc+I)@xr + bc ============
                tsb = []
                with tc.tile_pool(name="phc", bufs=NS) as phc, \
                     tc.tile_pool(name="phcs", bufs=4) as phcs, \
                     tc.tile_pool(name="phd", bufs=6) as phd:
                    xrs = []

                    def xr_sample(s):
                        # y = A*u + Cc (ACT) ; xr = af*x + y (fused DVE stt)
                        if s < len(u_pre):
                            us = u_pre[s]
                        else:
                            us = ut_pool.tile([P, FO, D], f16, tag="ut")
                            nc.sync.dma_start(us[:], u_dram[s][:])
                        xhs = xh_pool.tile([P, FO, D], f16, tag="xh")
                        nc.sync.dma_start(xhs[:], xh_io[s])
                        xr = phc.tile([P, FO, D], f16, tag="xr")
                        xrs.append(xr)
                        for fo in range(FO):
                            y = phcs.tile([P, D], f16, tag="y")
                            nc.scalar.activation(y[:], us[:, fo, :],
                                                 Ident, bias=Cc[:, fo:fo + 1],
                                                 scale=Abig[:, fo:fo + 1])
                            nc.vector.scalar_tensor_tensor(
                                out=xr[:, fo, :], in0=xhs[:, fo, :],
                                scalar=af[:, fo:fo + 1], in1=y[:],
                                op0=mult, op1=add)
                        ts = phc.tile([P, FO, D], f16, tag="ts")
                        tsb.append(ts)

                    def conv_sample(wave, s):
                        ocs = range(wave * 4, wave * 4 + 4)
                        pcs = {oc: pmm.tile([P, 512], f32, tag="mm",
                                            name=f"pc{oc}")
                               for oc in ocs}
                        for ic in range(FO):
                            for oc in ocs:
                                nc.tensor.matmul(
                                    pcs[oc][:],
                                    lhsT=WcIT[:, ic, oc * P:(oc + 1) * P],
                                    rhs=xrs[s][:, ic, :],
                                    start=(ic == 0), stop=(ic == FO - 1))
                        for oc in ocs:
                            nc.scalar.activation(tsb[s][:, oc, :],
                                                 pcs[oc][:], Ident,
                                                 bias=bc_col[:, oc:oc + 1],
                                                 scale=1.0)
                            nc.vector.bn_stats(out=tslots[:, oc, s, :],
                                               in_=tsb[s][:, oc, :])

                    def ar3_wave(wave):
                        # aggregate + AllReduce over this wave's 4 channels
                        # lanes: 0 S_t, 1 Q_t
                        oc0 = wave * 4
                        tsl = tslots[:, oc0:oc0 + 4, :, :]
                        ar3 = small.tile([P, 2, 4], f32, name=f"ar3_{wave}")
                        t4 = small.tile([P, 4, NS], f32, name=f"t4_{wave}")
                        t4b = small.tile([P, 4, NS], f32, name=f"t4b_{wave}")
                        tme = tsl[:, :, :, 1]
                        tmo = tsl[:, :, :, 4]
                        nc.vector.tensor_tensor(t4[:], tme, tmo, add)
                        nc.vector.tensor_reduce(out=ar3[:, 0, :], in_=t4[:],
                                                axis=AXX, op=add)
                        nc.vector.tensor_tensor(t4[:], tme, tme, mult)
                        nc.vector.tensor_tensor(t4b[:], tmo, tmo, mult)
                        nc.vector.tensor_tensor(t4[:], t4[:], t4b[:], add)
                        nc.vector.tensor_reduce(out=ar3[:, 1, :], in_=t4[:],
                                                axis=AXX, op=add)
                        nc.vector.tensor_tensor(t4[:], tsl[:, :, :, 2],
                                                tsl[:, :, :, 5], add)
                        nc.vector.tensor_reduce(out=t4b[:, :, 0:1], in_=t4[:],
                                                axis=AXX, op=add)
                        nc.vector.scalar_tensor_tensor(
                            out=ar3[:, 1, :], in0=ar3[:, 1, :], scalar=256.0,
                            in1=t4b[:, :, 0], op0=mult, op1=add)
                        ar3_in = dram.tile([P, 8], f32, tag=f"ar3{wave}_in")
                        ar3_out = dram.tile([P, 8], f32, tag=f"ar3{wave}_out")
                        nc.scalar.dma_start(
                            ar3_in[:], ar3[:].rearrange("p a b -> p (a b)"))
                        nc.gpsimd.collective_compute(
                            "AllReduce", add,
                            replica_groups=[list(range(NCORES))],
                            ins=[ar3_in.opt()], outs=[ar3_out.opt()])
                        return ar3_out

                    def gsb3_read(wave, ar3_out):
                        gsb3 = small.tile([P, 2, 4], f32, name=f"gsb3_{wave}")
                        nc.scalar.dma_start(
                            gsb3[:].rearrange("p a b -> p (a b)"), ar3_out[:])
                        return gsb3

                    def bn_affine_wave(wave, gsb3):
                        ocs = list(range(wave * 4, wave * 4 + 4))
                        mt = small.tile([P, 4], f32, name=f"mt{wave}")
                        e2t = small.tile([P, 4], f32, name=f"e2t{wave}")
                        nc.vector.tensor_scalar_mul(mt[:], gsb3[:, 0, :],
                                                    256.0 / (N * D))
                        nc.vector.tensor_scalar_mul(e2t[:], gsb3[:, 1, :],
                                                    1.0 / (N * D))
                        go4 = small.tile([P, 4], f32, name=f"go{wave}")
                        bo4 = small.tile([P, 4], f32, name=f"bo{wave}")
                        nc.vector.tensor_copy(out=go4[:],
                                              in_=gammao[:, ocs[0]:ocs[0] + 4])
                        nc.vector.tensor_copy(out=bo4[:],
                                              in_=betao[:, ocs[0]:ocs[0] + 4])
                        return affine_from(mt, e2t, go4, bo4, f"bno{wave}",
                                           w=4)

                    def bn_out_wave(wave, ao, co):
                        ocs = list(range(wave * 4, wave * 4 + 4))
                        for s in range(NS):
                            for k, fo in enumerate(ocs):
                                osb = phd.tile([P, D], f32, tag="osb")
                                if fo % 2 == 0:
                                    nc.scalar.activation(
                                        osb[:], tsb[s][:, fo, :], Ident,
                                        bias=co[:, k:k + 1],
                                        scale=ao[:, k:k + 1])
                                else:
                                    nc.vector.tensor_scalar(
                                        out=osb[:], in0=tsb[s][:, fo, :],
                                        scalar1=ao[:, k:k + 1],
                                        scalar2=co[:, k:k + 1],
                                        op0=mult, op1=add)
                                # wave 0 drains on Sync (during AR3b); wave 1
                                # alternates queues so its 4MB drains in half
                                # the time (Sync is idle again by then)
                                eng = (nc.sync if wave == 0 or
                                       (s + k) % 2 == 0 else nc.scalar)
                                eng.dma_start(
                                    out_io[s, fo * P:(fo + 1) * P, :], osb[:])

                    # interleave xr with wave-0 convs so PSUM-releasing bias
                    # ACTs aren't buried behind the xr ACT ops
                    for s in range(NS):
                        xr_sample(s)
                        if DEBUG and s == 0:
                            with tc.tile_pool(name="dbgx", bufs=1) as dbgx:
                                for fc in range(FO):
                                    st = dbgx.tile([P, D], f32, tag="st")
                                    nc.vector.tensor_copy(
                                        out=st[:], in_=xrs[0][:, fc, :])
                                    nc.sync.dma_start(dbg_xr[:, fc, :], st[:])
                        conv_sample(0, s)
                    ar3a = ar3_wave(0)
                    g3a = gsb3_read(0, ar3a)
                    conv_sample(1, 0)
                    conv_sample(1, 1)
                    # wave-0 affine: its DVE ops queue behind samples 0-1's
                    # wave-1 stats only, so they run mid-wave-1
                    ao0, co0 = bn_affine_wave(0, g3a)
                    conv_sample(1, 2)
                    conv_sample(1, 3)
                    ar3b = ar3_wave(1)   # trigger ASAP after s3 stats
                    g3b = gsb3_read(1, ar3b)
                    # wave-0 applies + output DMAs run during AR3b
                    bn_out_wave(0, ao0, co0)
                    ao1, co1 = bn_affine_wave(1, g3b)
                    bn_out_wave(1, ao1, co1)

    nc.compile()
    return nc


def _get_nc(has_bias=False):
    key = ("nc", has_bias)
    if key not in _CACHE:
        _CACHE[key] = _build(has_bias)
    return _CACHE[key]


def make_in_maps(inputs):
    """Host-side prep: shard x over cores, pre-transpose/cast weights."""
    x = np.ascontiguousarray(inputs["x"], dtype=np.float32)
    W0 = np.asarray(inputs["W0"], dtype=np.float32)
    W1 = np.asarray(inputs["W1"], dtype=np.float32)
    Wc = np.asarray(inputs["Wc"], dtype=np.float32)
    b0 = np.asarray(inputs["b0"], dtype=np.float32)
    b1 = np.asarray(inputs["b1"], dtype=np.float32)
    bc = np.asarray(inputs["bc"], dtype=np.float32)
    has_bias = bool(np.any(b0) or np.any(b1))

    # W0T[di, do, h] = W0[h, do*128+di]
    W0T = np.ascontiguousarray(
        W0.reshape(H, DO, P).transpose(2, 1, 0).astype(np.float16))
    # M[e, d] = (W1 @ W0)[d, e] -> z_tilde = x @ M ; MT[di, do, d]
    M = (W1 @ W0).T
    MT = np.ascontiguousarray(
        M.reshape(DO, P, D).transpose(1, 0, 2).astype(np.float16))
    # s1r[p, d] = sum_h W1[d, h], broadcast over partitions
    s1 = W1.sum(axis=1)
    s1r = np.ascontiguousarray(
        np.broadcast_to(s1[None, :], (P, D)).astype(np.float16))
    # WcIT[ii, io, o] = (Wc+I)[o, io*128+ii]
    WcI = Wc + np.eye(F, dtype=np.float32)
    WcIT = np.ascontiguousarray(
        WcI.reshape(F, FO, P).transpose(2, 1, 0).astype(np.float16))

    # prm [P, 9, FO]: g0, be0, g1, be1, gf, bf, go, bo, bc
    prm = np.stack([np.asarray(inputs[k], dtype=np.float32)
                    .reshape(FO, P).T for k in
                    ["g0", "be0", "g1", "be1", "gf", "bf", "go", "bo"]] +
                   [bc.reshape(FO, P).T], axis=1)
    prm = np.ascontiguousarray(prm)  # [P, 9, FO]

    shared = {"W0T": W0T, "MT": MT, "s1r": s1r, "WcIT": WcIT, "prm": prm}
    if has_bias:
        shared["b0r"] = np.ascontiguousarray(b0.reshape(1, H))
        shared["b1r"] = np.ascontiguousarray(b1.reshape(1, D))

    in_maps = []
    for c in range(NCORES):
        xs = x[c * NS:(c + 1) * NS]  # [NS, F, D]
        # xh[s, fi, fo, d] = x[s, fo*128+fi, d]
        xh = np.ascontiguousarray(
            xs.reshape(NS, FO, P, D).transpose(0, 2, 1, 3).astype(np.float16))
        # xT[s, di, do, f] = x[s, f, do*128+di]
        xT = np.ascontiguousarray(
            xs.reshape(NS, F, DO, P).transpose(0, 3, 2, 1).astype(np.float16))
        m = {"xh": xh, "xT": xT}
        m.update(shared)
        in_maps.append(m)
    return in_maps, has_bias


def kernel(**inputs) -> np.ndarray:
    from concourse import bass_utils

    in_maps, has_bias = make_in_maps(inputs)
    nc = _get_nc(has_bias)
    res = bass_utils.run_bass_kernel_spmd(
        nc, in_maps, core_ids=list(range(NCORES)), trace=False)
    out = np.concatenate([res.results[c]["out"] for c in range(NCORES)],
                         axis=0)
    return out.astype(np.float32)
